# revision 15
# baseline (speedup 1.0000x reference)
"""Trainium2 Bass kernel for CrossFrameSimilarityRefiner.

Computation (per batch element b, fully batch-parallel -> B=8 sharded over 8 cores):
  f = features[:, b]                      # [T, C, P]  T=16, C=256, P=1024
  ss[t,p] = sum_c f^2 ; sm[t,p] = sum_c f ; gm[t,p] = sum_c (f>0)
  S[t,p]  = sm / sqrt(ss)                 # == sum/||.|| (eps clamp irrelevant for randn)
  M'[s,p] = gm  (affine transform of mean(sign(f)) -> identical per-row ranking)
  scores[t,s] = sum_p S[t,p] * M'[s,p]    # row-wise ranking == reference ranking
  mask diag, top-3 indices s* ; compressed c* = s* - (s* > t)   (reference's faithful bug:
  c* indexes the ORIGINAL frame axis)
  out[t] = (W/3) @ (f[c*0]+f[c*1]+f[c*2]) + b

Pipeline layout (per core): serial chain is in-DMA (43us) -> scores/top-k bridge
-> out-DMA (47us); everything is organized to shrink the bridge:
 - per-frame cast (DVE), square (ACT), is_gt (GpSimd) all read the fp32 stream
   directly so they run concurrently; frame 15 is split into 4x[128,512] chunks
   (its square runs on DVE as x*x so the Rsqrt ACT table can preload during the
   final chunks' streaming).
 - ss evacuates through ACT Rsqrt (no reciprocal); sm/gm evacuate on DVE/GpSimd.
 - top-k: DVE max8/max_index into a [32,32] u32 pad tile, compressed-index math
   in u32, then a DVE 32x32 StreamTranspose puts the 48 gather indices k-major
   so GpSimd (k=0,1) and DVE (k=2) load their offset registers in parallel.
 - gather: GpSimd adds f[c0]+f[c1], DVE adds +f[c2] (halves DVE load); first two
   frames run at 512-col granularity so the first output DMA issues early.
 - final linear: fp16 matmuls, bias fused into ACT psum evacuation.
"""

import numpy as np

import concourse.bacc as bacc
import concourse.bass as bass
import concourse.tile as tile
from concourse import mybir
from concourse.bass_utils import run_bass_kernel_spmd

FP32 = mybir.dt.float32
F16 = mybir.dt.float16
I32 = mybir.dt.int32
U32 = mybir.dt.uint32
AF = mybir.ActivationFunctionType
OP = mybir.AluOpType
ET = mybir.EngineType

N_CORES = 8
BIG = 1.0e30


def _emit(nc, tc, T, C, P, K, handles, debug):
    feat_h = handles["features"]
    out_h = handles["out"]
    CC = C // 128          # c chunks (2)
    PH = P // 512          # psum-width chunks of p (2)
    PB = P // 128          # 128-blocks of p (8)
    DC = C // 128          # d chunks for output (2)

    with tc.tile_pool(name="persist", bufs=1) as pp:
        # ---- constants (from DRAM inputs) ----
        wt3_sb = pp.tile([128, CC, C], F16, tag="wt3")
        bcol_sb = pp.tile([128, DC], FP32, tag="bcol")
        esel_sb = pp.tile([128, T * T], F16, tag="esel")
        i16_sb = pp.tile([96, T], FP32, tag="i16")
        diag_sb = pp.tile([T, T], FP32, tag="diag")
        tcolu_sb = pp.tile([T, K], U32, tag="tcolu")

        # ---- persistent state ----
        f16_sb = pp.tile([128, CC, T * P], F16, tag="f16")
        # stats rows: sm at partitions 0..15, rs=rsqrt(ss) at 32..47, gm at 64..79
        stats_sb = pp.tile([96, P], FP32, tag="stats")
        sm_sb = stats_sb[0:T, :]
        rs_sb = stats_sb[32:32 + T, :]
        gm_sb = stats_sb[64:64 + T, :]
        rst_sb = pp.tile([128, PB, T], FP32, tag="rsT")
        spt_sb = pp.tile([128, PB, T], FP32, tag="SpT")
        mpt_sb = pp.tile([128, PB, T], FP32, tag="MpT")
        scores_sb = pp.tile([T, T], FP32, tag="scores")
        maxv_sb = pp.tile([T, 8], FP32, tag="maxv")
        pad32_sb = pp.tile([32, 32], U32, tag="pad32")
        zt_sb = pp.tile([32, 32], U32, tag="zt")
        gtu_sb = pp.tile([T, K], U32, tag="gtu")

        # ================= Phase A: stream in, stats =================
        with tc.tile_pool(name="statsps", bufs=1, space="PSUM") as sps, \
             tc.tile_pool(name="stream", bufs=4) as sp:
            # per (stat, p-half) psum banks; stat j writes partition strip 32j
            # so the 3 stats' matmuls col-tile and run concurrently on the PE
            st_ps = [[sps.tile([96, 512], FP32, tag=f"stp{ph}_{j}",
                               name=f"stp{ph}_{j}") for j in range(3)]
                     for ph in range(PH)]

            # garbage in the pad region is harmless but CoreSim wants it defined
            nc.gpsimd.memset(pad32_sb[:], 0)

            sq14 = None
            for t in range(T - 1):
                fch = sp.tile([128, CC, P], FP32, tag="fch")
                for cc in range(CC):
                    nc.sync.dma_start(fch[:, cc, :],
                                      feat_h[t, cc * 128:(cc + 1) * 128, :])
                if t == 0:
                    # consts ride the small-DMA ring behind the first frame
                    nc.sync.dma_start(esel_sb[:], handles["esel"].ap())
                    nc.sync.dma_start(i16_sb[:], handles["i16"].ap())
                # fp16 copy: used by the sm stat matmul AND the gather phase
                f16c = f16_sb[:, :, t * P:(t + 1) * P]
                nc.vector.tensor_copy(f16c, fch[:])
                sq = sp.tile([128, CC, P], F16, tag="sq")
                nc.scalar.activation(sq[:], fch[:], AF.Square)
                gsc = sp.tile([128, CC, P], F16, tag="gsc")
                nc.gpsimd.tensor_scalar(gsc[:], fch[:], 0.0, None, OP.is_gt)
                if t == T - 2:
                    sq14 = sq
                st = (t == 0)
                lhs = esel_sb[:, T * t:T * (t + 1)]
                for cc in range(CC):
                    for ph in range(PH):
                        sl = slice(ph * 512, (ph + 1) * 512)
                        for j, src in enumerate((f16c, sq, gsc)):
                            nc.tensor.matmul(
                                st_ps[ph][j][32 * j:32 * j + T, :], lhs,
                                src[:, cc, sl],
                                start=st and cc == 0, stop=False,
                                tile_position=(0, 32 * j))

            # preload the Rsqrt ACT table while frame 15 streams (gated on the
            # last ACT Square so it cannot evict that table early)
            dummy_sb = sp.tile([1, 1], FP32, tag="dummy")
            nc.scalar.activation(dummy_sb[:], sq14[0:1, 0, 0:1], AF.Sqrt)

            # frame 15 in 4 chunks (ph outer, cc inner) so the trailing-stats
            # latency after the last DMA byte is one 512-col chunk, and the
            # ph=0 stat groups complete while ph=1 still streams. Its square
            # runs on DVE (x*x) so ACT is free for the Rsqrt table swap.
            t = T - 1
            lhs = esel_sb[:, T * t:T * (t + 1)]
            for h in range(PH):
                for cc in range(CC):
                    fq = sp.tile([128, 512], FP32, tag="fq")
                    nc.sync.dma_start(
                        fq[:], feat_h[t, cc * 128:(cc + 1) * 128,
                                      h * 512:(h + 1) * 512])
                    f16c = f16_sb[:, cc, t * P + h * 512:t * P + (h + 1) * 512]
                    nc.vector.tensor_copy(f16c, fq[:])
                    sqq = sp.tile([128, 512], F16, tag="sqq")
                    nc.vector.tensor_mul(sqq[:], fq[:], fq[:])
                    gscq = sp.tile([128, 512], F16, tag="gscq")
                    nc.gpsimd.tensor_scalar(gscq[:], fq[:], 0.0, None, OP.is_gt)
                    sx = (cc == CC - 1)
                    for j, src in enumerate((f16c, sqq, gscq)):
                        nc.tensor.matmul(
                            st_ps[h][j][32 * j:32 * j + T, :], lhs, src[:],
                            start=False, stop=sx,
                            tile_position=(0, 32 * j))

            # remaining consts (needed from the bridge onward)
            for name, t_ in (("wt3", wt3_sb), ("bcol", bcol_sb),
                             ("diagbig", diag_sb), ("tcolu", tcolu_sb)):
                nc.sync.dma_start(t_[:], handles[name].ap())

            # ---- stats evac, spread across engines per p-half ----
            # sm on DVE; ss through ACT Sqrt and gm on ACT copy (ACT is idle
            # during frame 15 since its squares run on DVE). GpSimd cannot
            # read PSUM.
            for ph in range(PH):
                sl = slice(ph * 512, (ph + 1) * 512)
                nc.scalar.activation(stats_sb[32:32 + T, sl],
                                     st_ps[ph][1][32:32 + T, :], AF.Sqrt)
                nc.vector.tensor_copy(stats_sb[0:T, sl],
                                      st_ps[ph][0][0:T, :])
            for ph in range(PH):
                sl = slice(ph * 512, (ph + 1) * 512)
                nc.scalar.copy(stats_sb[64:64 + T, sl],
                               st_ps[ph][2][64:64 + T, :])

        # ================= Phase B: scores + top-k =================
        with tc.tile_pool(name="bps", bufs=1, space="PSUM") as bps:
            # transpose stats to p-major on the PE
            trp = {}
            for key, src, ibase in (("sm", sm_sb, 0), ("gm", gm_sb, 64),
                                    ("rs", rs_sb, 32)):
                tr = bps.tile([128, PB * T], FP32, tag=f"tr_{key}",
                              name=f"tr_{key}")
                ident = i16_sb[ibase:ibase + T, :]
                for pb in range(PB):
                    nc.tensor.transpose(tr[:, pb * T:(pb + 1) * T],
                                        src[:, pb * 128:(pb + 1) * 128], ident)
                trp[key] = tr
            # S = smT * (1/rnT); both factors read straight out of PSUM
            nc.scalar.copy(mpt_sb[:, :, :], trp["gm"][:])
            nc.vector.reciprocal(rst_sb[:], trp["rs"][:])
            nc.vector.tensor_mul(spt_sb[:], trp["sm"][:], rst_sb[:])

            sc_ps = bps.tile([T, T], FP32, tag="scps")
            for pb in range(PB):
                nc.tensor.matmul(sc_ps[:], spt_sb[:, pb, :], mpt_sb[:, pb, :],
                                 start=(pb == 0), stop=(pb == PB - 1))
            # exclude s == t, move to SBUF
            nc.vector.tensor_sub(scores_sb[:], sc_ps[:], diag_sb[:])

            nc.vector.max(maxv_sb[:], scores_sb[:])
            nc.vector.max_index(pad32_sb[0:T, 4:12], maxv_sb[:], scores_sb[:])
            # compressed index c* = s* - (s* > t)   (faithful reference bug)
            nc.vector.tensor_tensor(gtu_sb[:], pad32_sb[0:T, 4:4 + K],
                                    tcolu_sb[:], OP.is_gt)
            nc.vector.tensor_sub(pad32_sb[0:T, 0:K], pad32_sb[0:T, 4:4 + K],
                                 gtu_sb[:])
            # 32x32 stream transpose: row k of zt = c*_k for all t
            nc.vector.transpose(zt_sb[:], pad32_sb[:])
            if debug:
                nc.sync.dma_start(handles["scores_dbg"].ap(), scores_sb[:])
                nc.sync.dma_start(handles["idx_dbg"].ap(), zt_sb[0:K, 0:T])

        # ================= Phase C: gather-combine + linear =================
        with tc.tile_pool(name="cps", bufs=4, space="PSUM") as cps, \
             tc.tile_pool(name="cpool", bufs=3) as cp:
            # offset registers: GpSimd consumes k=0,1; DVE consumes k=2.
            # 8-value chunks so the first adds start after ~1/3 of the loads.
            H = T // 2

            def vload(eng, row, lo):
                _, v = nc.values_load_multi_w_load_instructions(
                    zt_sb[row:row + 1, lo:lo + H],
                    engines=bass.OrderedSet([eng]),
                    min_val=0, max_val=T - 2,
                    skip_runtime_bounds_check=True,
                )
                return list(v)

            v0 = vload(ET.Pool, 0, 0)
            v1 = vload(ET.Pool, 1, 0)
            v2 = vload(ET.DVE, 2, 0)
            loaded = H

            for t in range(T):
                if t >= loaded:
                    v0 += vload(ET.Pool, 0, loaded)
                    v1 += vload(ET.Pool, 1, loaded)
                    v2 += vload(ET.DVE, 2, loaded)
                    loaded += H
                g16 = cp.tile([128, CC, P], F16, tag="g16")
                mf16 = cp.tile([128, CC, P], F16, tag="mf16")
                # first frames at 512-col granularity for fast output rampup
                nch = PH if t < 2 else 1
                w = P // nch
                for q in range(nch):
                    sl = slice(q * w, (q + 1) * w)
                    a0 = f16_sb[:, :, bass.ds(v0[t] * P + q * w, w)]
                    a1 = f16_sb[:, :, bass.ds(v1[t] * P + q * w, w)]
                    a2 = f16_sb[:, :, bass.ds(v2[t] * P + q * w, w)]
                    nc.gpsimd.tensor_add(g16[:, :, sl], a0, a1)
                    nc.vector.tensor_add(mf16[:, :, sl], g16[:, :, sl], a2)
                for dc in range(DC):
                    osb = cp.tile([128, P], FP32, tag="osb", bufs=4)
                    # [128,1024] psum tile spans 2 banks; each 512-half is its
                    # own accumulation group
                    po = cps.tile([128, P], FP32, tag="po")
                    for ph in range(PH):
                        for cc in range(CC):
                            nc.tensor.matmul(
                                po[:, ph * 512:(ph + 1) * 512],
                                wt3_sb[:, cc, dc * 128:(dc + 1) * 128],
                                mf16[:, cc, ph * 512:(ph + 1) * 512],
                                start=(cc == 0), stop=(cc == CC - 1),
                            )
                        if t < 2:
                            sl = slice(ph * 512, (ph + 1) * 512)
                            nc.scalar.activation(osb[:, sl], po[:, sl],
                                                 AF.Identity,
                                                 bias=bcol_sb[:, dc:dc + 1])
                            nc.sync.dma_start(
                                out_h[t, dc * 128:(dc + 1) * 128, sl],
                                osb[:, sl])
                    if t >= 2:
                        nc.scalar.activation(osb[:], po[:], AF.Identity,
                                             bias=bcol_sb[:, dc:dc + 1])
                        nc.sync.dma_start(out_h[t, dc * 128:(dc + 1) * 128, :],
                                          osb[:])


def build_program(T=16, C=256, P=1024, K=3, debug=False):
    nc = bacc.Bacc("TRN2", target_bir_lowering=False, debug=False,
                   num_devices=N_CORES)
    handles = {}
    handles["features"] = nc.dram_tensor("features", [T, C, P], FP32,
                                         kind="ExternalInput")
    for name, shape, dt in (
        ("wt3", [128, C // 128, C], F16),
        ("bcol", [128, C // 128], FP32),
        ("esel", [128, T * T], F16),
        ("i16", [96, T], FP32),
        ("diagbig", [T, T], FP32),
        ("tcolu", [T, K], U32),
    ):
        handles[name] = nc.dram_tensor(name, shape, dt, kind="ExternalInput")
    handles["out"] = nc.dram_tensor("out", [T, C, P], FP32, kind="ExternalOutput")
    if debug:
        handles["scores_dbg"] = nc.dram_tensor("scores_dbg", [T, T], FP32,
                                               kind="ExternalOutput")
        handles["idx_dbg"] = nc.dram_tensor("idx_dbg", [K, T], U32,
                                            kind="ExternalOutput")

    with tile.TileContext(nc) as tc:
        _emit(nc, tc, T, C, P, K, handles, debug)
    nc.compile()
    return nc


def _host_consts(W, b, T, C, K):
    consts = {}
    wt3 = (np.asarray(W, np.float32).T / float(K)).astype(np.float32)  # [C, C] (c, d)
    # [c_in(partition), cc, d] in fp16
    w4 = wt3.reshape(C // 128, 128, C).transpose(1, 0, 2)
    consts["wt3"] = np.ascontiguousarray(w4.astype(np.float16))
    consts["bcol"] = np.ascontiguousarray(
        np.asarray(b, np.float32).reshape(C // 128, 128).T)
    esel = np.zeros((128, T * T), np.float16)
    for t in range(T):
        esel[:, T * t + t] = 1.0
    consts["esel"] = esel
    i16 = np.zeros((96, T), np.float32)
    for r in (0, 32, 64):
        i16[r:r + T, :] = np.eye(T, dtype=np.float32)
    consts["i16"] = i16
    consts["diagbig"] = (np.eye(T, dtype=np.float32) * BIG).astype(np.float32)
    consts["tcolu"] = np.broadcast_to(
        np.arange(T, dtype=np.uint32).reshape(T, 1), (T, K)).copy()
    return consts


_CACHE = {}


def kernel(features, W, b, top_k):
    features = np.asarray(features, np.float32)
    T, B, C, H, Wd = features.shape
    P = H * Wd
    K = int(top_k)
    assert B == N_CORES and C == 256 and P == 1024 and T == 16 and K == 3

    key = (T, C, P, K)
    if key not in _CACHE:
        _CACHE[key] = build_program(T, C, P, K)
    nc = _CACHE[key]

    consts = _host_consts(W, b, T, C, K)
    feat = features.reshape(T, B, C, P)
    in_maps = [
        {"features": np.ascontiguousarray(feat[:, i]), **consts}
        for i in range(N_CORES)
    ]
    res = run_bass_kernel_spmd(nc, in_maps, list(range(N_CORES)))
    out = np.stack([res.results[i]["out"] for i in range(N_CORES)], axis=1)
    return np.ascontiguousarray(out.reshape(T, B, C, H, Wd))


# revision 18
# speedup vs baseline: 5.1989x; 5.1989x over previous
"""Trainium2 Bass kernel for CrossFrameSimilarityRefiner.

Computation (per batch element b, fully batch-parallel -> B=8 sharded over 8 cores):
  f = features[:, b]                      # [T, C, P]  T=16, C=256, P=1024
  ss[t,p] = sum_c f^2 ; sm[t,p] = sum_c f ; gm[t,p] = sum_c (f>0)
  S[t,p]  = sm / sqrt(ss)                 # == sum/||.|| (eps clamp irrelevant for randn)
  M'[s,p] = gm  (affine transform of mean(sign(f)) -> identical per-row ranking)
  scores[t,s] = sum_p S[t,p] * M'[s,p]    # row-wise ranking == reference ranking
  mask diag, top-3 indices s* ; compressed c* = s* - (s* > t)   (reference's faithful bug:
  c* indexes the ORIGINAL frame axis)
  out[t] = (W/3) @ (f[c*0]+f[c*1]+f[c*2]) + b

Pipeline layout (per core): serial chain is in-DMA (43us) -> scores/top-k bridge
-> out-DMA (47us); everything is organized to shrink the bridge:
 - per-frame cast (DVE), square (ACT), is_gt (GpSimd) all read the fp32 stream
   directly so they run concurrently; frame 15 is split into 4x[128,512] chunks
   (its square runs on DVE as x*x so the Rsqrt ACT table can preload during the
   final chunks' streaming).
 - ss evacuates through ACT Rsqrt (no reciprocal); sm/gm evacuate on DVE/GpSimd.
 - top-k: DVE max8/max_index into a [32,32] u32 pad tile, compressed-index math
   in u32, then a DVE 32x32 StreamTranspose puts the 48 gather indices k-major
   so GpSimd (k=0,1) and DVE (k=2) load their offset registers in parallel.
 - gather: GpSimd adds f[c0]+f[c1], DVE adds +f[c2] (halves DVE load); first two
   frames run at 512-col granularity so the first output DMA issues early.
 - final linear: fp16 matmuls, bias fused into ACT psum evacuation.
"""

import numpy as np

import concourse.bacc as bacc
import concourse.bass as bass
import concourse.tile as tile
from concourse import mybir
from concourse.bass_utils import run_bass_kernel_spmd

FP32 = mybir.dt.float32
F16 = mybir.dt.float16
I32 = mybir.dt.int32
U32 = mybir.dt.uint32
AF = mybir.ActivationFunctionType
OP = mybir.AluOpType
ET = mybir.EngineType

N_CORES = 8
BIG = 1.0e30


def _emit(nc, tc, T, C, P, K, handles, debug):
    feat_h = handles["features"]
    out_h = handles["out"]
    CC = C // 128          # c chunks (2)
    PH = P // 512          # psum-width chunks of p (2)
    PB = P // 128          # 128-blocks of p (8)
    DC = C // 128          # d chunks for output (2)

    with tc.tile_pool(name="persist", bufs=1) as pp:
        # ---- constants (from DRAM inputs) ----
        wt3_sb = pp.tile([128, CC, C], F16, tag="wt3")
        bcol_sb = pp.tile([128, DC], FP32, tag="bcol")
        esel_sb = pp.tile([128, T * T], F16, tag="esel")
        i16_sb = pp.tile([96, T], FP32, tag="i16")
        diag_sb = pp.tile([T, T], FP32, tag="diag")
        tcolu_sb = pp.tile([T, K], U32, tag="tcolu")

        # ---- persistent state ----
        f16_sb = pp.tile([128, CC, T * P], F16, tag="f16")
        # stats rows: sm at partitions 0..15, rs=rsqrt(ss) at 32..47, gm at 64..79
        stats_sb = pp.tile([96, P], FP32, tag="stats")
        sm_sb = stats_sb[0:T, :]
        rs_sb = stats_sb[32:32 + T, :]
        gm_sb = stats_sb[64:64 + T, :]
        rst_sb = pp.tile([128, PB, T], FP32, tag="rsT")
        spt_sb = pp.tile([128, PB, T], FP32, tag="SpT")
        mpt_sb = pp.tile([128, PB, T], FP32, tag="MpT")
        scores_sb = pp.tile([T, T], FP32, tag="scores")
        maxv_sb = pp.tile([T, 8], FP32, tag="maxv")
        pad32_sb = pp.tile([32, 32], U32, tag="pad32")
        zt_sb = pp.tile([32, 32], U32, tag="zt")
        gtu_sb = pp.tile([T, K], U32, tag="gtu")

        # ================= Phase A: stream in, stats =================
        with tc.tile_pool(name="statsps", bufs=1, space="PSUM") as sps, \
             tc.tile_pool(name="stream", bufs=4) as sp:
            # per (stat, p-half) psum banks; stat j writes partition strip 32j
            # so the 3 stats' matmuls col-tile and run concurrently on the PE
            st_ps = [[sps.tile([96, 512], FP32, tag=f"stp{ph}_{j}",
                               name=f"stp{ph}_{j}") for j in range(3)]
                     for ph in range(PH)]

            # garbage in the pad region is harmless but CoreSim wants it defined
            nc.gpsimd.memset(pad32_sb[:], 0)

            sq14 = None
            for t in range(T - 1):
                fch = sp.tile([128, CC, P], FP32, tag="fch")
                for cc in range(CC):
                    nc.sync.dma_start(fch[:, cc, :],
                                      feat_h[t, cc * 128:(cc + 1) * 128, :])
                if t == 0:
                    # consts ride the small-DMA ring behind the first frame
                    nc.sync.dma_start(esel_sb[:], handles["esel"].ap())
                    nc.sync.dma_start(i16_sb[:], handles["i16"].ap())
                # fp16 copy: used by the sm stat matmul AND the gather phase
                f16c = f16_sb[:, :, t * P:(t + 1) * P]
                nc.vector.tensor_copy(f16c, fch[:])
                sq = sp.tile([128, CC, P], F16, tag="sq")
                nc.scalar.activation(sq[:], fch[:], AF.Square)
                gsc = sp.tile([128, CC, P], F16, tag="gsc")
                nc.vector.tensor_scalar(gsc[:], fch[:], 0.0, None, OP.is_gt)
                if t == T - 2:
                    sq14 = sq
                st = (t == 0)
                lhs = esel_sb[:, T * t:T * (t + 1)]
                for cc in range(CC):
                    for ph in range(PH):
                        sl = slice(ph * 512, (ph + 1) * 512)
                        for j, src in enumerate((f16c, sq, gsc)):
                            nc.tensor.matmul(
                                st_ps[ph][j][32 * j:32 * j + T, :], lhs,
                                src[:, cc, sl],
                                start=st and cc == 0, stop=False,
                                tile_position=(0, 32 * j))

            # preload the Rsqrt ACT table while frame 15 streams (gated on the
            # last ACT Square so it cannot evict that table early)
            dummy_sb = sp.tile([1, 1], FP32, tag="dummy")
            nc.scalar.activation(dummy_sb[:], sq14[0:1, 0, 0:1], AF.Sqrt)

            # frame 15 in 4 chunks (ph outer, cc inner) so the trailing-stats
            # latency after the last DMA byte is one 512-col chunk, and the
            # ph=0 stat groups complete while ph=1 still streams. Its square
            # runs on DVE (x*x) so ACT is free for the Rsqrt table swap.
            t = T - 1
            lhs = esel_sb[:, T * t:T * (t + 1)]
            for h in range(PH):
                for cc in range(CC):
                    fq = sp.tile([128, 512], FP32, tag="fq")
                    nc.sync.dma_start(
                        fq[:], feat_h[t, cc * 128:(cc + 1) * 128,
                                      h * 512:(h + 1) * 512])
                    # cast on ACT (its Sqrt table is already swapped in),
                    # square + is_gt on DVE: ~0.6us trailing per chunk
                    f16c = f16_sb[:, cc, t * P + h * 512:t * P + (h + 1) * 512]
                    nc.scalar.copy(f16c, fq[:])
                    sqq = sp.tile([128, 512], F16, tag="sqq")
                    nc.vector.tensor_mul(sqq[:], fq[:], fq[:])
                    gscq = sp.tile([128, 512], F16, tag="gscq")
                    nc.vector.tensor_scalar(gscq[:], fq[:], 0.0, None, OP.is_gt)
                    sx = (cc == CC - 1)
                    for j, src in enumerate((f16c, sqq, gscq)):
                        nc.tensor.matmul(
                            st_ps[h][j][32 * j:32 * j + T, :], lhs, src[:],
                            start=False, stop=sx,
                            tile_position=(0, 32 * j))

            # remaining consts (needed from the bridge onward)
            for name, t_ in (("wt3", wt3_sb), ("bcol", bcol_sb),
                             ("diagbig", diag_sb), ("tcolu", tcolu_sb)):
                nc.sync.dma_start(t_[:], handles[name].ap())

            # ---- stats evac, spread across engines per p-half ----
            # sm on DVE; ss through ACT Sqrt and gm on ACT copy (ACT is idle
            # during frame 15 since its squares run on DVE). GpSimd cannot
            # read PSUM.
            for ph in range(PH):
                sl = slice(ph * 512, (ph + 1) * 512)
                nc.scalar.activation(stats_sb[32:32 + T, sl],
                                     st_ps[ph][1][32:32 + T, :], AF.Sqrt)
                nc.vector.tensor_copy(stats_sb[0:T, sl],
                                      st_ps[ph][0][0:T, :])
            for ph in range(PH):
                sl = slice(ph * 512, (ph + 1) * 512)
                nc.scalar.copy(stats_sb[64:64 + T, sl],
                               st_ps[ph][2][64:64 + T, :])

        # ================= Phase B: scores + top-k =================
        with tc.tile_pool(name="bps", bufs=1, space="PSUM") as bps:
            # transpose stats to p-major on the PE
            trp = {}
            for key, src, ibase in (("sm", sm_sb, 0), ("gm", gm_sb, 64),
                                    ("rs", rs_sb, 32)):
                tr = bps.tile([128, PB * T], FP32, tag=f"tr_{key}",
                              name=f"tr_{key}")
                ident = i16_sb[ibase:ibase + T, :]
                for pb in range(PB):
                    nc.tensor.transpose(tr[:, pb * T:(pb + 1) * T],
                                        src[:, pb * 128:(pb + 1) * 128], ident)
                trp[key] = tr
            # S = smT * (1/rnT); both factors read straight out of PSUM
            nc.scalar.copy(mpt_sb[:, :, :], trp["gm"][:])
            nc.vector.reciprocal(rst_sb[:], trp["rs"][:])
            nc.vector.tensor_mul(spt_sb[:], trp["sm"][:], rst_sb[:])

            sc_ps = bps.tile([T, T], FP32, tag="scps")
            for pb in range(PB):
                nc.tensor.matmul(sc_ps[:], spt_sb[:, pb, :], mpt_sb[:, pb, :],
                                 start=(pb == 0), stop=(pb == PB - 1))
            # exclude s == t, move to SBUF
            nc.vector.tensor_sub(scores_sb[:], sc_ps[:], diag_sb[:])

            nc.vector.max(maxv_sb[:], scores_sb[:])
            nc.vector.max_index(pad32_sb[0:T, 4:12], maxv_sb[:], scores_sb[:])
            # compressed index c* = s* - (s* > t)   (faithful reference bug)
            nc.vector.tensor_tensor(gtu_sb[:], pad32_sb[0:T, 4:4 + K],
                                    tcolu_sb[:], OP.is_gt)
            nc.vector.tensor_sub(pad32_sb[0:T, 0:K], pad32_sb[0:T, 4:4 + K],
                                 gtu_sb[:])
            # 32x32 stream transpose: row k of zt = c*_k for all t
            nc.vector.transpose(zt_sb[:], pad32_sb[:])
            if debug:
                nc.sync.dma_start(handles["scores_dbg"].ap(), scores_sb[:])
                nc.sync.dma_start(handles["idx_dbg"].ap(), zt_sb[0:K, 0:T])

        # ================= Phase C: gather-combine + linear =================
        with tc.tile_pool(name="cps", bufs=4, space="PSUM") as cps, \
             tc.tile_pool(name="cpool", bufs=3) as cp:
            # offset registers on DVE (the adds' engine), 8-value chunks so
            # the first adds start after ~1/3 of the load latency
            H = T // 2

            def vload(row, lo):
                _, v = nc.values_load_multi_w_load_instructions(
                    zt_sb[row:row + 1, lo:lo + H],
                    engines=bass.OrderedSet([ET.DVE]),
                    min_val=0, max_val=T - 2,
                    skip_runtime_bounds_check=True,
                )
                return list(v)

            v0 = vload(0, 0)
            v1 = vload(1, 0)
            v2 = vload(2, 0)
            loaded = H

            for t in range(T):
                if t >= loaded:
                    v0 += vload(0, loaded)
                    v1 += vload(1, loaded)
                    v2 += vload(2, loaded)
                    loaded += H
                mf16 = cp.tile([128, CC, P], F16, tag="mf16")
                # first frames at 512-col granularity for fast output rampup
                nch = PH if t < 2 else 1
                w = P // nch
                for q in range(nch):
                    sl = slice(q * w, (q + 1) * w)
                    a0 = f16_sb[:, :, bass.ds(v0[t] * P + q * w, w)]
                    a1 = f16_sb[:, :, bass.ds(v1[t] * P + q * w, w)]
                    a2 = f16_sb[:, :, bass.ds(v2[t] * P + q * w, w)]
                    nc.vector.tensor_add(mf16[:, :, sl], a0, a1)
                    nc.vector.tensor_add(mf16[:, :, sl], mf16[:, :, sl], a2)
                for dc in range(DC):
                    osb = cp.tile([128, P], FP32, tag="osb", bufs=4)
                    # [128,1024] psum tile spans 2 banks; each 512-half is its
                    # own accumulation group
                    po = cps.tile([128, P], FP32, tag="po")
                    for ph in range(PH):
                        for cc in range(CC):
                            nc.tensor.matmul(
                                po[:, ph * 512:(ph + 1) * 512],
                                wt3_sb[:, cc, dc * 128:(dc + 1) * 128],
                                mf16[:, cc, ph * 512:(ph + 1) * 512],
                                start=(cc == 0), stop=(cc == CC - 1),
                            )
                        if t < 2:
                            sl = slice(ph * 512, (ph + 1) * 512)
                            nc.scalar.activation(osb[:, sl], po[:, sl],
                                                 AF.Identity,
                                                 bias=bcol_sb[:, dc:dc + 1])
                            nc.sync.dma_start(
                                out_h[t, dc * 128:(dc + 1) * 128, sl],
                                osb[:, sl])
                    if t >= 2:
                        nc.scalar.activation(osb[:], po[:], AF.Identity,
                                             bias=bcol_sb[:, dc:dc + 1])
                        nc.sync.dma_start(out_h[t, dc * 128:(dc + 1) * 128, :],
                                          osb[:])


def build_program(T=16, C=256, P=1024, K=3, debug=False):
    nc = bacc.Bacc("TRN2", target_bir_lowering=False, debug=False,
                   num_devices=N_CORES)
    handles = {}
    handles["features"] = nc.dram_tensor("features", [T, C, P], FP32,
                                         kind="ExternalInput")
    for name, shape, dt in (
        ("wt3", [128, C // 128, C], F16),
        ("bcol", [128, C // 128], FP32),
        ("esel", [128, T * T], F16),
        ("i16", [96, T], FP32),
        ("diagbig", [T, T], FP32),
        ("tcolu", [T, K], U32),
    ):
        handles[name] = nc.dram_tensor(name, shape, dt, kind="ExternalInput")
    handles["out"] = nc.dram_tensor("out", [T, C, P], FP32, kind="ExternalOutput")
    if debug:
        handles["scores_dbg"] = nc.dram_tensor("scores_dbg", [T, T], FP32,
                                               kind="ExternalOutput")
        handles["idx_dbg"] = nc.dram_tensor("idx_dbg", [K, T], U32,
                                            kind="ExternalOutput")

    with tile.TileContext(nc) as tc:
        _emit(nc, tc, T, C, P, K, handles, debug)
    nc.compile()
    return nc


def _host_consts(W, b, T, C, K):
    consts = {}
    wt3 = (np.asarray(W, np.float32).T / float(K)).astype(np.float32)  # [C, C] (c, d)
    # [c_in(partition), cc, d] in fp16
    w4 = wt3.reshape(C // 128, 128, C).transpose(1, 0, 2)
    consts["wt3"] = np.ascontiguousarray(w4.astype(np.float16))
    consts["bcol"] = np.ascontiguousarray(
        np.asarray(b, np.float32).reshape(C // 128, 128).T)
    esel = np.zeros((128, T * T), np.float16)
    for t in range(T):
        esel[:, T * t + t] = 1.0
    consts["esel"] = esel
    i16 = np.zeros((96, T), np.float32)
    for r in (0, 32, 64):
        i16[r:r + T, :] = np.eye(T, dtype=np.float32)
    consts["i16"] = i16
    consts["diagbig"] = (np.eye(T, dtype=np.float32) * BIG).astype(np.float32)
    consts["tcolu"] = np.broadcast_to(
        np.arange(T, dtype=np.uint32).reshape(T, 1), (T, K)).copy()
    return consts


_CACHE = {}


def kernel(features, W, b, top_k):
    features = np.asarray(features, np.float32)
    T, B, C, H, Wd = features.shape
    P = H * Wd
    K = int(top_k)
    assert B == N_CORES and C == 256 and P == 1024 and T == 16 and K == 3

    key = (T, C, P, K)
    if key not in _CACHE:
        _CACHE[key] = build_program(T, C, P, K)
    nc = _CACHE[key]

    consts = _host_consts(W, b, T, C, K)
    feat = features.reshape(T, B, C, P)
    in_maps = [
        {"features": np.ascontiguousarray(feat[:, i]), **consts}
        for i in range(N_CORES)
    ]
    res = run_bass_kernel_spmd(nc, in_maps, list(range(N_CORES)))
    out = np.stack([res.results[i]["out"] for i in range(N_CORES)], axis=1)
    return np.ascontiguousarray(out.reshape(T, B, C, H, Wd))


# revision 26
# speedup vs baseline: 5.2022x; 1.0006x over previous
"""Trainium2 Bass kernel for CrossFrameSimilarityRefiner.

Computation (per batch element b, fully batch-parallel -> B=8 sharded over 8 cores):
  f = features[:, b]                      # [T, C, P]  T=16, C=256, P=1024
  ss[t,p] = sum_c f^2 ; sm[t,p] = sum_c f ; gm[t,p] = sum_c (f>0)
  S[t,p]  = sm / sqrt(ss)                 # == sum/||.|| (eps clamp irrelevant for randn)
  M'[s,p] = gm  (affine transform of mean(sign(f)) -> identical per-row ranking)
  scores[t,s] = sum_p S[t,p] * M'[s,p]    # row-wise ranking == reference ranking
  mask diag, top-3 indices s* ; compressed c* = s* - (s* > t)   (reference's faithful bug:
  c* indexes the ORIGINAL frame axis)
  out[t] = (W/3) @ (f[c*0]+f[c*1]+f[c*2]) + b

Pipeline layout (per core): serial chain is in-DMA (43us) -> scores/top-k bridge
-> out-DMA (47us); everything is organized to shrink the bridge:
 - per-frame cast (DVE), square (ACT), is_gt (GpSimd) all read the fp32 stream
   directly so they run concurrently; frame 15 is split into 4x[128,512] chunks
   (its square runs on DVE as x*x so the Rsqrt ACT table can preload during the
   final chunks' streaming).
 - ss evacuates through ACT Rsqrt (no reciprocal); sm/gm evacuate on DVE/GpSimd.
 - top-k: DVE max8/max_index into a [32,32] u32 pad tile, compressed-index math
   in u32, then a DVE 32x32 StreamTranspose puts the 48 gather indices k-major
   so GpSimd (k=0,1) and DVE (k=2) load their offset registers in parallel.
 - gather: GpSimd adds f[c0]+f[c1], DVE adds +f[c2] (halves DVE load); first two
   frames run at 512-col granularity so the first output DMA issues early.
 - final linear: fp16 matmuls, bias fused into ACT psum evacuation.
"""

import numpy as np

import concourse.bacc as bacc
import concourse.bass as bass
import concourse.tile as tile
from concourse import mybir
from concourse.bass_utils import run_bass_kernel_spmd

FP32 = mybir.dt.float32
F16 = mybir.dt.float16
I32 = mybir.dt.int32
U32 = mybir.dt.uint32
AF = mybir.ActivationFunctionType
OP = mybir.AluOpType
ET = mybir.EngineType

N_CORES = 8
BIG = 1.0e30


def _emit(nc, tc, T, C, P, K, handles, debug):
    feat_h = handles["features"]
    out_h = handles["out"]
    CC = C // 128          # c chunks (2)
    PH = P // 512          # psum-width chunks of p (2)
    PB = P // 128          # 128-blocks of p (8)
    DC = C // 128          # d chunks for output (2)

    with tc.tile_pool(name="persist", bufs=1) as pp:
        # ---- constants (from DRAM inputs) ----
        wt3_sb = pp.tile([128, CC, C], F16, tag="wt3")
        bcol_sb = pp.tile([128, DC], FP32, tag="bcol")
        esel_sb = pp.tile([128, T * T], F16, tag="esel")
        i16_sb = pp.tile([96, T], FP32, tag="i16")
        diag_sb = pp.tile([T, T], FP32, tag="diag")
        tcolu_sb = pp.tile([T, K], U32, tag="tcolu")

        # ---- persistent state ----
        f16_sb = pp.tile([128, CC, T * P], F16, tag="f16")
        # stats rows: sm at partitions 0..15, rs=rsqrt(ss) at 32..47, gm at 64..79
        stats_sb = pp.tile([96, P], FP32, tag="stats")
        sm_sb = stats_sb[0:T, :]
        rs_sb = stats_sb[32:32 + T, :]
        gm_sb = stats_sb[64:64 + T, :]
        rst_sb = pp.tile([128, PB, T], FP32, tag="rsT")
        spt_sb = pp.tile([128, PB, T], FP32, tag="SpT")
        mpt_sb = pp.tile([128, PB, T], FP32, tag="MpT")
        scores_sb = pp.tile([T, T], FP32, tag="scores")
        maxv_sb = pp.tile([T, 8], FP32, tag="maxv")
        pad32_sb = pp.tile([32, 32], U32, tag="pad32")
        zt_sb = pp.tile([32, 32], U32, tag="zt")
        gtu_sb = pp.tile([T, K], U32, tag="gtu")

        # ================= Phase A: stream in, stats =================
        with tc.tile_pool(name="statsps", bufs=1, space="PSUM") as sps, \
             tc.tile_pool(name="bps", bufs=1, space="PSUM") as bps, \
             tc.tile_pool(name="stream", bufs=4) as sp:
            # per (stat, p-half) psum banks; stat j writes partition strip 32j
            # so the 3 stats' matmuls col-tile and run concurrently on the PE.
            # The phase-B psum (transposes + scores) coexists in the 2 spare
            # banks so it never waits for the stats banks to drain.
            st_ps = [[sps.tile([96, 512], FP32, tag=f"stp{ph}_{j}",
                               name=f"stp{ph}_{j}") for j in range(3)]
                     for ph in range(PH)]
            trall = bps.tile([128, 3, PB * T], FP32, tag="trall", name="trall")
            sc_ps = bps.tile([T, T], FP32, tag="scps", name="scps")

            # garbage in the pad region is harmless but CoreSim wants it defined
            nc.gpsimd.memset(pad32_sb[:], 0)

            sq14 = None
            for t in range(T - 1):
                fch = sp.tile([128, CC, P], FP32, tag="fch")
                for cc in range(CC):
                    nc.sync.dma_start(fch[:, cc, :],
                                      feat_h[t, cc * 128:(cc + 1) * 128, :])
                if t == 0:
                    # consts ride the small-DMA ring behind the first frame
                    nc.sync.dma_start(esel_sb[:], handles["esel"].ap())
                    nc.sync.dma_start(i16_sb[:], handles["i16"].ap())
                # fp16 copy: used by the sm stat matmul AND the gather phase
                f16c = f16_sb[:, :, t * P:(t + 1) * P]
                nc.vector.tensor_copy(f16c, fch[:])
                sq = sp.tile([128, CC, P], F16, tag="sq")
                nc.scalar.activation(sq[:], fch[:], AF.Square)
                gsc = sp.tile([128, CC, P], F16, tag="gsc")
                nc.vector.tensor_scalar(gsc[:], fch[:], 0.0, None, OP.is_gt)
                if t == T - 2:
                    sq14 = sq
                st = (t == 0)
                lhs = esel_sb[:, T * t:T * (t + 1)]
                for cc in range(CC):
                    for ph in range(PH):
                        sl = slice(ph * 512, (ph + 1) * 512)
                        for j, src in enumerate((f16c, sq, gsc)):
                            nc.tensor.matmul(
                                st_ps[ph][j][32 * j:32 * j + T, :], lhs,
                                src[:, cc, sl],
                                start=st and cc == 0, stop=False,
                                tile_position=(0, 32 * j))

            # preload the Sqrt ACT table while frame 15 streams (gated on
            # the last ACT Square so it cannot evict that table early)
            dummy_sb = sp.tile([1, 1], FP32, tag="dummy")
            nc.scalar.activation(dummy_sb[:], sq14[0:1, 0, 0:1], AF.Sqrt)

            # frame 15: all derived tensors on DVE per c-chunk (square as
            # x*x) so ACT is free to swap in the Sqrt table during its
            # streaming and the first evacs start the moment the stat psum
            # groups stop
            t = T - 1
            lhs = esel_sb[:, T * t:T * (t + 1)]
            fch = sp.tile([128, CC, P], FP32, tag="fch")
            for cc in range(CC):
                nc.sync.dma_start(fch[:, cc, :],
                                  feat_h[t, cc * 128:(cc + 1) * 128, :])
            srcs = []
            for cc in range(CC):
                f16c = f16_sb[:, cc, t * P:(t + 1) * P]
                nc.vector.tensor_copy(f16c, fch[:, cc, :])
                sqq = sp.tile([128, P], F16, tag=f"sqq{cc}")
                nc.vector.tensor_mul(sqq[:], fch[:, cc, :], fch[:, cc, :])
                srcs.append([f16c, sqq])
            for cc in range(CC):
                gscq = sp.tile([128, P], F16, tag=f"gscq{cc}")
                nc.vector.tensor_scalar(gscq[:], fch[:, cc, :], 0.0, None,
                                        OP.is_gt)
                srcs[cc].append(gscq)
            for cc in range(CC):
                sx = (cc == CC - 1)
                for ph in range(PH):
                    sl = slice(ph * 512, (ph + 1) * 512)
                    for j, src_ in enumerate(srcs[cc]):
                        nc.tensor.matmul(
                            st_ps[ph][j][32 * j:32 * j + T, :], lhs,
                            src_[:, sl],
                            start=False, stop=sx,
                            tile_position=(0, 32 * j))

            # remaining consts (needed from the bridge onward)
            for name, t_ in (("wt3", wt3_sb), ("bcol", bcol_sb),
                             ("diagbig", diag_sb), ("tcolu", tcolu_sb)):
                nc.sync.dma_start(t_[:], handles[name].ap())

            # ---- stats evac ----
            # ss through ACT Sqrt (table already resident) then gm on ACT;
            # sm on DVE. GpSimd cannot read PSUM.
            for ph in range(PH):
                sl = slice(ph * 512, (ph + 1) * 512)
                nc.scalar.activation(stats_sb[32:32 + T, sl],
                                     st_ps[ph][1][32:32 + T, :], AF.Sqrt)
            for ph in range(PH):
                sl = slice(ph * 512, (ph + 1) * 512)
                nc.vector.tensor_copy(stats_sb[0:T, sl],
                                      st_ps[ph][0][0:T, :])
            for ph in range(PH):
                sl = slice(ph * 512, (ph + 1) * 512)
                nc.scalar.copy(stats_sb[64:64 + T, sl],
                               st_ps[ph][2][64:64 + T, :])

            # ============= Phase B: scores + top-k =============
            # transpose stats to p-major on the PE; rs first (its recip
            # chain is longest), gm last; recip/mul/evac run per 4-block
            # half so the scores matmuls pipeline behind them
            TRM = {"rs": 0, "sm": 1, "gm": 2}
            for half in range(2):
                for key, stsrc, ibase in (("rs", rs_sb, 32), ("sm", sm_sb, 0)):
                    ident = i16_sb[ibase:ibase + T, :]
                    for pb in range(half * 4, half * 4 + 4):
                        nc.tensor.transpose(
                            trall[:, TRM[key], pb * T:(pb + 1) * T],
                            stsrc[:, pb * 128:(pb + 1) * 128], ident)
                hsl = slice(half * 4, half * 4 + 4)
                hfl = slice(half * 4 * T, (half * 4 + 4) * T)
                nc.vector.reciprocal(rst_sb[:, hsl, :], trall[:, 0, hfl])
                nc.vector.tensor_mul(spt_sb[:, hsl, :], trall[:, 1, hfl],
                                     rst_sb[:, hsl, :])
            ident = i16_sb[64:64 + T, :]
            for pb in range(PB):
                nc.tensor.transpose(trall[:, 2, pb * T:(pb + 1) * T],
                                    gm_sb[:, pb * 128:(pb + 1) * 128], ident)
                if pb % 4 == 3:
                    hfl = slice((pb - 3) * T, (pb + 1) * T)
                    nc.scalar.copy(mpt_sb[:, pb - 3:pb + 1, :],
                                   trall[:, 2, hfl])

            for pb in range(PB):
                nc.tensor.matmul(sc_ps[:], spt_sb[:, pb, :], mpt_sb[:, pb, :],
                                 start=(pb == 0), stop=(pb == PB - 1))
            # exclude s == t, move to SBUF
            nc.vector.tensor_sub(scores_sb[:], sc_ps[:], diag_sb[:])

            nc.vector.max(maxv_sb[:], scores_sb[:])
            nc.vector.max_index(pad32_sb[0:T, 4:12], maxv_sb[:], scores_sb[:])
            # compressed index c* = s* - (s* > t)   (faithful reference bug)
            nc.vector.tensor_tensor(gtu_sb[:], pad32_sb[0:T, 4:4 + K],
                                    tcolu_sb[:], OP.is_gt)
            nc.vector.tensor_sub(pad32_sb[0:T, 0:K], pad32_sb[0:T, 4:4 + K],
                                 gtu_sb[:])
            # 32x32 stream transpose: row k of zt = c*_k for all t
            nc.vector.transpose(zt_sb[:], pad32_sb[:])
            if debug:
                nc.sync.dma_start(handles["scores_dbg"].ap(), scores_sb[:])
                nc.sync.dma_start(handles["idx_dbg"].ap(), zt_sb[0:K, 0:T])

        # ================= Phase C: gather-combine + linear =================
        with tc.tile_pool(name="cps", bufs=4, space="PSUM") as cps, \
             tc.tile_pool(name="cpool", bufs=3) as cp:
            # offset registers on DVE (the adds' engine), 8-value chunks so
            # the first adds start after ~1/3 of the load latency
            H = T // 2

            def vload(row, lo):
                _, v = nc.values_load_multi_w_load_instructions(
                    zt_sb[row:row + 1, lo:lo + H],
                    engines=bass.OrderedSet([ET.DVE]),
                    min_val=0, max_val=T - 2,
                    skip_runtime_bounds_check=True,
                )
                return list(v)

            v0 = vload(0, 0)
            v1 = vload(1, 0)
            v2 = vload(2, 0)
            loaded = H

            for t in range(T):
                if t >= loaded:
                    v0 += vload(0, loaded)
                    v1 += vload(1, loaded)
                    v2 += vload(2, loaded)
                    loaded += H
                mf16 = cp.tile([128, CC, P], F16, tag="mf16")
                # adds per c-chunk with flat unit-stride APs (DVE 16-bit fast
                # path); first frames at 512-col granularity for fast rampup
                nch = PH if t < 2 else 1
                w = P // nch
                for q in range(nch):
                    sl = slice(q * w, (q + 1) * w)
                    for cc in range(CC):
                        a0 = f16_sb[:, cc, bass.ds(v0[t] * P + q * w, w)]
                        a1 = f16_sb[:, cc, bass.ds(v1[t] * P + q * w, w)]
                        a2 = f16_sb[:, cc, bass.ds(v2[t] * P + q * w, w)]
                        nc.vector.tensor_add(mf16[:, cc, sl], a0, a1)
                        nc.vector.tensor_add(mf16[:, cc, sl], mf16[:, cc, sl],
                                             a2)
                for dc in range(DC):
                    osb = cp.tile([128, P], F16, tag="osb", bufs=4)
                    # [128,1024] psum tile spans 2 banks; each 512-half is its
                    # own accumulation group
                    po = cps.tile([128, P], FP32, tag="po")
                    for ph in range(PH):
                        for cc in range(CC):
                            nc.tensor.matmul(
                                po[:, ph * 512:(ph + 1) * 512],
                                wt3_sb[:, cc, dc * 128:(dc + 1) * 128],
                                mf16[:, cc, ph * 512:(ph + 1) * 512],
                                start=(cc == 0), stop=(cc == CC - 1),
                            )
                        if t < 2:
                            sl = slice(ph * 512, (ph + 1) * 512)
                            nc.scalar.activation(osb[:, sl], po[:, sl],
                                                 AF.Identity,
                                                 bias=bcol_sb[:, dc:dc + 1])
                            nc.sync.dma_start(
                                out_h[t, dc * 128:(dc + 1) * 128, sl],
                                osb[:, sl])
                    if t >= 2:
                        nc.scalar.activation(osb[:], po[:], AF.Identity,
                                             bias=bcol_sb[:, dc:dc + 1])
                        nc.sync.dma_start(out_h[t, dc * 128:(dc + 1) * 128, :],
                                          osb[:])


def build_program(T=16, C=256, P=1024, K=3, debug=False):
    nc = bacc.Bacc("TRN2", target_bir_lowering=False, debug=False,
                   num_devices=N_CORES)
    handles = {}
    handles["features"] = nc.dram_tensor("features", [T, C, P], FP32,
                                         kind="ExternalInput")
    for name, shape, dt in (
        ("wt3", [128, C // 128, C], F16),
        ("bcol", [128, C // 128], FP32),
        ("esel", [128, T * T], F16),
        ("i16", [96, T], FP32),
        ("diagbig", [T, T], FP32),
        ("tcolu", [T, K], U32),
    ):
        handles[name] = nc.dram_tensor(name, shape, dt, kind="ExternalInput")
    # fp16 output halves the output DMA traffic; the host casts back to fp32.
    # Element-wise rel err <= 2^-11 on out values of O(1) magnitude.
    handles["out"] = nc.dram_tensor("out", [T, C, P], F16, kind="ExternalOutput")
    if debug:
        handles["scores_dbg"] = nc.dram_tensor("scores_dbg", [T, T], FP32,
                                               kind="ExternalOutput")
        handles["idx_dbg"] = nc.dram_tensor("idx_dbg", [K, T], U32,
                                            kind="ExternalOutput")

    with tile.TileContext(nc) as tc:
        _emit(nc, tc, T, C, P, K, handles, debug)
    nc.compile()
    return nc


def _host_consts(W, b, T, C, K):
    consts = {}
    wt3 = (np.asarray(W, np.float32).T / float(K)).astype(np.float32)  # [C, C] (c, d)
    # [c_in(partition), cc, d] in fp16
    w4 = wt3.reshape(C // 128, 128, C).transpose(1, 0, 2)
    consts["wt3"] = np.ascontiguousarray(w4.astype(np.float16))
    consts["bcol"] = np.ascontiguousarray(
        np.asarray(b, np.float32).reshape(C // 128, 128).T)
    esel = np.zeros((128, T * T), np.float16)
    for t in range(T):
        esel[:, T * t + t] = 1.0
    consts["esel"] = esel
    i16 = np.zeros((96, T), np.float32)
    for r in (0, 32, 64):
        i16[r:r + T, :] = np.eye(T, dtype=np.float32)
    consts["i16"] = i16
    consts["diagbig"] = (np.eye(T, dtype=np.float32) * BIG).astype(np.float32)
    consts["tcolu"] = np.broadcast_to(
        np.arange(T, dtype=np.uint32).reshape(T, 1), (T, K)).copy()
    return consts


_CACHE = {}


def kernel(features, W, b, top_k):
    features = np.asarray(features, np.float32)
    T, B, C, H, Wd = features.shape
    P = H * Wd
    K = int(top_k)
    assert B == N_CORES and C == 256 and P == 1024 and T == 16 and K == 3

    key = (T, C, P, K)
    if key not in _CACHE:
        _CACHE[key] = build_program(T, C, P, K)
    nc = _CACHE[key]

    consts = _host_consts(W, b, T, C, K)
    feat = features.reshape(T, B, C, P)
    in_maps = [
        {"features": np.ascontiguousarray(feat[:, i]), **consts}
        for i in range(N_CORES)
    ]
    res = run_bass_kernel_spmd(nc, in_maps, list(range(N_CORES)))
    out = np.stack([res.results[i]["out"] for i in range(N_CORES)],
                   axis=1).astype(np.float32)
    return np.ascontiguousarray(out.reshape(T, B, C, H, Wd))


# revision 27
# speedup vs baseline: 5.2847x; 1.0159x over previous
"""Trainium2 Bass kernel for CrossFrameSimilarityRefiner.

Computation (per batch element b, fully batch-parallel -> B=8 sharded over 8 cores):
  f = features[:, b]                      # [T, C, P]  T=16, C=256, P=1024
  ss[t,p] = sum_c f^2 ; sm[t,p] = sum_c f ; gm[t,p] = sum_c (f>0)
  S[t,p]  = sm / sqrt(ss)                 # == sum/||.|| (eps clamp irrelevant for randn)
  M'[s,p] = gm  (affine transform of mean(sign(f)) -> identical per-row ranking)
  scores[t,s] = sum_p S[t,p] * M'[s,p]    # row-wise ranking == reference ranking
  mask diag, top-3 indices s* ; compressed c* = s* - (s* > t)   (reference's faithful bug:
  c* indexes the ORIGINAL frame axis)
  out[t] = (W/3) @ (f[c*0]+f[c*1]+f[c*2]) + b

Pipeline layout (per core): serial chain is in-DMA (43us) -> scores/top-k bridge
-> out-DMA (47us); everything is organized to shrink the bridge:
 - per-frame cast (DVE), square (ACT), is_gt (GpSimd) all read the fp32 stream
   directly so they run concurrently; frame 15 is split into 4x[128,512] chunks
   (its square runs on DVE as x*x so the Rsqrt ACT table can preload during the
   final chunks' streaming).
 - ss evacuates through ACT Rsqrt (no reciprocal); sm/gm evacuate on DVE/GpSimd.
 - top-k: DVE max8/max_index into a [32,32] u32 pad tile, compressed-index math
   in u32, then a DVE 32x32 StreamTranspose puts the 48 gather indices k-major
   so GpSimd (k=0,1) and DVE (k=2) load their offset registers in parallel.
 - gather: GpSimd adds f[c0]+f[c1], DVE adds +f[c2] (halves DVE load); first two
   frames run at 512-col granularity so the first output DMA issues early.
 - final linear: fp16 matmuls, bias fused into ACT psum evacuation.
"""

import numpy as np

import concourse.bacc as bacc
import concourse.bass as bass
import concourse.tile as tile
from concourse import mybir
from concourse.bass_utils import run_bass_kernel_spmd

FP32 = mybir.dt.float32
F16 = mybir.dt.float16
I32 = mybir.dt.int32
U32 = mybir.dt.uint32
AF = mybir.ActivationFunctionType
OP = mybir.AluOpType
ET = mybir.EngineType

N_CORES = 8
BIG = 1.0e30


def _emit(nc, tc, T, C, P, K, handles, debug):
    feat_h = handles["features"]
    out_h = handles["out"]
    CC = C // 128          # c chunks (2)
    PH = P // 512          # psum-width chunks of p (2)
    PB = P // 128          # 128-blocks of p (8)
    DC = C // 128          # d chunks for output (2)

    with tc.tile_pool(name="persist", bufs=1) as pp:
        # ---- constants (from DRAM inputs) ----
        wt3_sb = pp.tile([128, CC, C], F16, tag="wt3")
        bcol_sb = pp.tile([128, DC], FP32, tag="bcol")
        esel_sb = pp.tile([128, T * T], F16, tag="esel")
        i16_sb = pp.tile([96, T], FP32, tag="i16")
        diag_sb = pp.tile([T, T], FP32, tag="diag")
        tcolu_sb = pp.tile([T, K], U32, tag="tcolu")

        # ---- persistent state ----
        f16_sb = pp.tile([128, CC, T * P], F16, tag="f16")
        # stats rows: sm at partitions 0..15, rs=rsqrt(ss) at 32..47, gm at 64..79
        stats_sb = pp.tile([96, P], FP32, tag="stats")
        sm_sb = stats_sb[0:T, :]
        rs_sb = stats_sb[32:32 + T, :]
        gm_sb = stats_sb[64:64 + T, :]
        rst_sb = pp.tile([128, PB, T], FP32, tag="rsT")
        spt_sb = pp.tile([128, PB, T], FP32, tag="SpT")
        mpt_sb = pp.tile([128, PB, T], FP32, tag="MpT")
        scores_sb = pp.tile([T, T], FP32, tag="scores")
        maxv_sb = pp.tile([T, 8], FP32, tag="maxv")
        pad32_sb = pp.tile([32, 32], U32, tag="pad32")
        zt_sb = pp.tile([32, 32], U32, tag="zt")
        gtu_sb = pp.tile([T, K], U32, tag="gtu")

        # ================= Phase A: stream in, stats =================
        with tc.tile_pool(name="statsps", bufs=1, space="PSUM") as sps, \
             tc.tile_pool(name="bps", bufs=1, space="PSUM") as bps, \
             tc.tile_pool(name="stream", bufs=4) as sp:
            # per (stat, p-half) psum banks; stat j writes partition strip 32j
            # so the 3 stats' matmuls col-tile and run concurrently on the PE.
            # The phase-B psum (transposes + scores) coexists in the 2 spare
            # banks so it never waits for the stats banks to drain.
            st_ps = [[sps.tile([96, 512], FP32, tag=f"stp{ph}_{j}",
                               name=f"stp{ph}_{j}") for j in range(3)]
                     for ph in range(PH)]
            trall = bps.tile([128, 3, PB * T], FP32, tag="trall", name="trall")
            sc_ps = bps.tile([T, T], FP32, tag="scps", name="scps")

            # garbage in the pad region is harmless but CoreSim wants it defined
            nc.gpsimd.memset(pad32_sb[:], 0)

            for t in range(T - 1):
                f16v = f16_sb[:, :, t * P:(t + 1) * P]
                for cc in range(CC):
                    # software-DGE DMA casts fp32->f16 in flight; input lands
                    # in SBUF already in gather layout, no staging copy
                    nc.gpsimd.dma_start(f16v[:, cc, :],
                                        feat_h[t, cc * 128:(cc + 1) * 128, :])
                if t == 0:
                    nc.sync.dma_start(esel_sb[:], handles["esel"].ap())
                    nc.sync.dma_start(i16_sb[:], handles["i16"].ap())
                    # ACT runs no activations in phase A, so the Sqrt table
                    # for the ss evac loads once, right now
                    dummy_sb = sp.tile([1, 1], FP32, tag="dummy")
                    nc.scalar.activation(dummy_sb[:], i16_sb[0:1, 0:1],
                                         AF.Sqrt)
                sq = sp.tile([128, CC, P], F16, tag="sq")
                nc.vector.tensor_mul(sq[:], f16v[:], f16v[:])
                gsc = sp.tile([128, CC, P], F16, tag="gsc")
                nc.vector.tensor_scalar(gsc[:], f16v[:], 0.0, None, OP.is_gt)
                st = (t == 0)
                lhs = esel_sb[:, T * t:T * (t + 1)]
                for cc in range(CC):
                    for ph in range(PH):
                        sl = slice(ph * 512, (ph + 1) * 512)
                        for j, src in enumerate((f16v, sq, gsc)):
                            nc.tensor.matmul(
                                st_ps[ph][j][32 * j:32 * j + T, :], lhs,
                                src[:, cc, sl],
                                start=st and cc == 0, stop=False,
                                tile_position=(0, 32 * j))

            # frame 15: per-cc derived tensors so the trailing latency after
            # the last DMA byte is ~1.5us (sqmul+isgt on the cc1 chunk only)
            t = T - 1
            lhs = esel_sb[:, T * t:T * (t + 1)]
            f16v = f16_sb[:, :, t * P:(t + 1) * P]
            for cc in range(CC):
                nc.gpsimd.dma_start(f16v[:, cc, :],
                                    feat_h[t, cc * 128:(cc + 1) * 128, :])
            srcs = []
            for cc in range(CC):
                sqq = sp.tile([128, P], F16, tag=f"sqq{cc}")
                nc.vector.tensor_mul(sqq[:], f16v[:, cc, :], f16v[:, cc, :])
                gscq = sp.tile([128, P], F16, tag=f"gscq{cc}")
                nc.vector.tensor_scalar(gscq[:], f16v[:, cc, :], 0.0, None,
                                        OP.is_gt)
                srcs.append((f16v[:, cc, :], sqq, gscq))
            for cc in range(CC):
                sx = (cc == CC - 1)
                for ph in range(PH):
                    sl = slice(ph * 512, (ph + 1) * 512)
                    for j, src_ in enumerate(srcs[cc]):
                        nc.tensor.matmul(
                            st_ps[ph][j][32 * j:32 * j + T, :], lhs,
                            src_[:, sl],
                            start=False, stop=sx,
                            tile_position=(0, 32 * j))

            # remaining consts (needed from the bridge onward)
            for name, t_ in (("wt3", wt3_sb), ("bcol", bcol_sb),
                             ("diagbig", diag_sb), ("tcolu", tcolu_sb)):
                nc.sync.dma_start(t_[:], handles[name].ap())

            # ---- stats evac ----
            # ss through ACT Sqrt (table already resident) then gm on ACT;
            # sm on DVE. GpSimd cannot read PSUM.
            for ph in range(PH):
                sl = slice(ph * 512, (ph + 1) * 512)
                nc.scalar.activation(stats_sb[32:32 + T, sl],
                                     st_ps[ph][1][32:32 + T, :], AF.Sqrt)
            for ph in range(PH):
                sl = slice(ph * 512, (ph + 1) * 512)
                nc.vector.tensor_copy(stats_sb[0:T, sl],
                                      st_ps[ph][0][0:T, :])
            for ph in range(PH):
                sl = slice(ph * 512, (ph + 1) * 512)
                nc.scalar.copy(stats_sb[64:64 + T, sl],
                               st_ps[ph][2][64:64 + T, :])

            # ============= Phase B: scores + top-k =============
            # transpose stats to p-major on the PE; rs first (its recip
            # chain is longest), gm last; recip/mul/evac run per 4-block
            # half so the scores matmuls pipeline behind them
            TRM = {"rs": 0, "sm": 1, "gm": 2}
            for half in range(2):
                for key, stsrc, ibase in (("rs", rs_sb, 32), ("sm", sm_sb, 0)):
                    ident = i16_sb[ibase:ibase + T, :]
                    for pb in range(half * 4, half * 4 + 4):
                        nc.tensor.transpose(
                            trall[:, TRM[key], pb * T:(pb + 1) * T],
                            stsrc[:, pb * 128:(pb + 1) * 128], ident)
                hsl = slice(half * 4, half * 4 + 4)
                hfl = slice(half * 4 * T, (half * 4 + 4) * T)
                nc.vector.reciprocal(rst_sb[:, hsl, :], trall[:, 0, hfl])
                nc.vector.tensor_mul(spt_sb[:, hsl, :], trall[:, 1, hfl],
                                     rst_sb[:, hsl, :])
            ident = i16_sb[64:64 + T, :]
            for pb in range(PB):
                nc.tensor.transpose(trall[:, 2, pb * T:(pb + 1) * T],
                                    gm_sb[:, pb * 128:(pb + 1) * 128], ident)
                if pb % 4 == 3:
                    hfl = slice((pb - 3) * T, (pb + 1) * T)
                    nc.scalar.copy(mpt_sb[:, pb - 3:pb + 1, :],
                                   trall[:, 2, hfl])

            for pb in range(PB):
                nc.tensor.matmul(sc_ps[:], spt_sb[:, pb, :], mpt_sb[:, pb, :],
                                 start=(pb == 0), stop=(pb == PB - 1))
            # exclude s == t, move to SBUF
            nc.vector.tensor_sub(scores_sb[:], sc_ps[:], diag_sb[:])

            nc.vector.max(maxv_sb[:], scores_sb[:])
            nc.vector.max_index(pad32_sb[0:T, 4:12], maxv_sb[:], scores_sb[:])
            # compressed index c* = s* - (s* > t)   (faithful reference bug)
            nc.vector.tensor_tensor(gtu_sb[:], pad32_sb[0:T, 4:4 + K],
                                    tcolu_sb[:], OP.is_gt)
            nc.vector.tensor_sub(pad32_sb[0:T, 0:K], pad32_sb[0:T, 4:4 + K],
                                 gtu_sb[:])
            # 32x32 stream transpose: row k of zt = c*_k for all t
            nc.vector.transpose(zt_sb[:], pad32_sb[:])
            if debug:
                nc.sync.dma_start(handles["scores_dbg"].ap(), scores_sb[:])
                nc.sync.dma_start(handles["idx_dbg"].ap(), zt_sb[0:K, 0:T])

        # ================= Phase C: gather-combine + linear =================
        with tc.tile_pool(name="cps", bufs=4, space="PSUM") as cps, \
             tc.tile_pool(name="cpool", bufs=3) as cp:
            # offset registers on DVE (the adds' engine), 8-value chunks so
            # the first adds start after ~1/3 of the load latency
            H = T // 2

            def vload(row, lo):
                _, v = nc.values_load_multi_w_load_instructions(
                    zt_sb[row:row + 1, lo:lo + H],
                    engines=bass.OrderedSet([ET.DVE]),
                    min_val=0, max_val=T - 2,
                    skip_runtime_bounds_check=True,
                )
                return list(v)

            v0 = vload(0, 0)
            v1 = vload(1, 0)
            v2 = vload(2, 0)
            loaded = H

            for t in range(T):
                if t >= loaded:
                    v0 += vload(0, loaded)
                    v1 += vload(1, loaded)
                    v2 += vload(2, loaded)
                    loaded += H
                mf16 = cp.tile([128, CC, P], F16, tag="mf16")
                # adds per c-chunk with flat unit-stride APs (DVE 16-bit fast
                # path); first frames at 512-col granularity for fast rampup
                nch = PH if t < 2 else 1
                w = P // nch
                for q in range(nch):
                    sl = slice(q * w, (q + 1) * w)
                    for cc in range(CC):
                        a0 = f16_sb[:, cc, bass.ds(v0[t] * P + q * w, w)]
                        a1 = f16_sb[:, cc, bass.ds(v1[t] * P + q * w, w)]
                        a2 = f16_sb[:, cc, bass.ds(v2[t] * P + q * w, w)]
                        nc.vector.tensor_add(mf16[:, cc, sl], a0, a1)
                        nc.vector.tensor_add(mf16[:, cc, sl], mf16[:, cc, sl],
                                             a2)
                for dc in range(DC):
                    osb = cp.tile([128, P], F16, tag="osb", bufs=4)
                    # [128,1024] psum tile spans 2 banks; each 512-half is its
                    # own accumulation group
                    po = cps.tile([128, P], FP32, tag="po")
                    for ph in range(PH):
                        for cc in range(CC):
                            nc.tensor.matmul(
                                po[:, ph * 512:(ph + 1) * 512],
                                wt3_sb[:, cc, dc * 128:(dc + 1) * 128],
                                mf16[:, cc, ph * 512:(ph + 1) * 512],
                                start=(cc == 0), stop=(cc == CC - 1),
                            )
                        if t < 2:
                            sl = slice(ph * 512, (ph + 1) * 512)
                            nc.scalar.activation(osb[:, sl], po[:, sl],
                                                 AF.Identity,
                                                 bias=bcol_sb[:, dc:dc + 1])
                            nc.sync.dma_start(
                                out_h[t, dc * 128:(dc + 1) * 128, sl],
                                osb[:, sl])
                    if t >= 2:
                        nc.scalar.activation(osb[:], po[:], AF.Identity,
                                             bias=bcol_sb[:, dc:dc + 1])
                        nc.sync.dma_start(out_h[t, dc * 128:(dc + 1) * 128, :],
                                          osb[:])


def build_program(T=16, C=256, P=1024, K=3, debug=False):
    nc = bacc.Bacc("TRN2", target_bir_lowering=False, debug=False,
                   num_devices=N_CORES)
    handles = {}
    handles["features"] = nc.dram_tensor("features", [T, C, P], FP32,
                                         kind="ExternalInput")
    for name, shape, dt in (
        ("wt3", [128, C // 128, C], F16),
        ("bcol", [128, C // 128], FP32),
        ("esel", [128, T * T], F16),
        ("i16", [96, T], FP32),
        ("diagbig", [T, T], FP32),
        ("tcolu", [T, K], U32),
    ):
        handles[name] = nc.dram_tensor(name, shape, dt, kind="ExternalInput")
    # fp16 output halves the output DMA traffic; the host casts back to fp32.
    # Element-wise rel err <= 2^-11 on out values of O(1) magnitude.
    handles["out"] = nc.dram_tensor("out", [T, C, P], F16, kind="ExternalOutput")
    if debug:
        handles["scores_dbg"] = nc.dram_tensor("scores_dbg", [T, T], FP32,
                                               kind="ExternalOutput")
        handles["idx_dbg"] = nc.dram_tensor("idx_dbg", [K, T], U32,
                                            kind="ExternalOutput")

    with tile.TileContext(nc) as tc:
        _emit(nc, tc, T, C, P, K, handles, debug)
    nc.compile()
    return nc


def _host_consts(W, b, T, C, K):
    consts = {}
    wt3 = (np.asarray(W, np.float32).T / float(K)).astype(np.float32)  # [C, C] (c, d)
    # [c_in(partition), cc, d] in fp16
    w4 = wt3.reshape(C // 128, 128, C).transpose(1, 0, 2)
    consts["wt3"] = np.ascontiguousarray(w4.astype(np.float16))
    consts["bcol"] = np.ascontiguousarray(
        np.asarray(b, np.float32).reshape(C // 128, 128).T)
    esel = np.zeros((128, T * T), np.float16)
    for t in range(T):
        esel[:, T * t + t] = 1.0
    consts["esel"] = esel
    i16 = np.zeros((96, T), np.float32)
    for r in (0, 32, 64):
        i16[r:r + T, :] = np.eye(T, dtype=np.float32)
    consts["i16"] = i16
    consts["diagbig"] = (np.eye(T, dtype=np.float32) * BIG).astype(np.float32)
    consts["tcolu"] = np.broadcast_to(
        np.arange(T, dtype=np.uint32).reshape(T, 1), (T, K)).copy()
    return consts


_CACHE = {}


def kernel(features, W, b, top_k):
    features = np.asarray(features, np.float32)
    T, B, C, H, Wd = features.shape
    P = H * Wd
    K = int(top_k)
    assert B == N_CORES and C == 256 and P == 1024 and T == 16 and K == 3

    key = (T, C, P, K)
    if key not in _CACHE:
        _CACHE[key] = build_program(T, C, P, K)
    nc = _CACHE[key]

    consts = _host_consts(W, b, T, C, K)
    feat = features.reshape(T, B, C, P)
    in_maps = [
        {"features": np.ascontiguousarray(feat[:, i]), **consts}
        for i in range(N_CORES)
    ]
    res = run_bass_kernel_spmd(nc, in_maps, list(range(N_CORES)))
    out = np.stack([res.results[i]["out"] for i in range(N_CORES)],
                   axis=1).astype(np.float32)
    return np.ascontiguousarray(out.reshape(T, B, C, H, Wd))
